# revision 5
# baseline (speedup 1.0000x reference)
"""Trainium2 Bass kernel for ColorHistogramLoss.

Reference computation:
  brightness = mean(target, axis=1)           # [B,1,H,W]
  mask = brightness > 0.4
  soft 16-bin Gaussian histograms of pred/target per (b, c), masked,
  normalized; loss = mean |pred_hist - target_hist|.

Kernel strategy (8 NeuronCores, data-parallel over batch B=8):
  Each core processes one image pair (pred[b], target[b]) [3,512,512].

  Per-bin weight exp(-128*(x-c_k)^2) is evaluated with the ScalarE
  Derivative_Erf activation, which is an exact Gaussian:
      Derivative_Erf(z) = (2/sqrt(pi)) * exp(-z^2)
  so one ScalarE op per bin computes the weight AND its free-axis sum
  (fused accum_out) from x' = x + mask_offset, with the affine
  z = sqrt(128)*x' - sqrt(128)*c_k folded into the activation's
  scale/bias.  The constant 2/sqrt(pi) is divided out on host.

  The mask is additive: x' = x - 4 where masked out, so every bin's
  Gaussian is exactly 0 there (z <= -34; exp underflows to 0 in f32).

  To beat ScalarE's 16-passes floor, bins 1..7 are produced on the DVE
  with the exact factorization
      G(x-c_k) = G(x-c_{k-1}) * w * beta_k,
      w = exp((256/15)x'),  beta_k = exp(-128(2k-1)/225)
  one fused scalar_tensor_tensor multiply per bin off the ScalarE seed
  G(x-c_0), with the bin sum from STT's own fused accum_out.  (The
  dedicated tensor_tensor_reduce instruction faults this runtime; STT's
  accum path works.)

  Layout: one channel per tile, [128, 2048] (262144 = 128*2048), so the
  brightness mask is a plain elementwise op over target tiles with no
  cross-partition shuffling.  Channel order: t0,t1,t2,p0,p1,p2.

  Per-channel engine budget (F=2048): ScalarE 10 ops (w, seed, 8 direct
  bins) ~20us; DVE x' + 7 chain STTs ~18.6us.  ScalarE-bound ~120us
  steady state vs ~238us for the previous all-ScalarE-exp kernel whose
  DVE built per-bin exponents.

  Output per core: stats [128, 96] per-partition accums for ScalarE
  bins (k=0 and 8..15), dstats [128, 42] chain-bin accums; the tiny
  normalize / L1 / mean finish runs on host.
"""

from contextlib import ExitStack

import numpy as np

import concourse.bass as bass
import concourse.tile as tile
from concourse import bacc, mybir
from concourse.bass_utils import run_bass_kernel_spmd

N_CORES = 8
C = 3
H = 512
W = 512
HW = H * W          # 262144
P = 128
FP = HW // P        # 2048
NB = 16
NCH = 6             # t0,t1,t2,p0,p1,p2
L_CHAIN = 7         # bins 1..L_CHAIN via DVE chain
SQ128 = 11.313708498984761   # sqrt(128)
W_SCALE = 256.0 / 15.0
OFF = -4.0
F32 = mybir.dt.float32
GAUSS_CONST = 1.1283791670955126  # 2/sqrt(pi)

BETA = [float(np.exp(-128.0 * (2 * k - 1) / 225.0)) for k in range(NB)]
# DVE emission slot (within the 7-step chain) at which the next
# channel's x' is built, so ScalarE's next w never waits.
XP_POS = 4


def _kernel_body(ctx: ExitStack, tc: "tile.TileContext", stats_d, dstats_d,
                 pred_d, target_d, repeat=1):
    nc = tc.nc
    rawp = ctx.enter_context(tc.tile_pool(name="rawp", bufs=1))
    maskp = ctx.enter_context(tc.tile_pool(name="maskp", bufs=1))
    xpool = ctx.enter_context(tc.tile_pool(name="xpool", bufs=2))
    wpool = ctx.enter_context(tc.tile_pool(name="wpool", bufs=2))
    fpool = ctx.enter_context(tc.tile_pool(name="fpool", bufs=3))
    ppool = ctx.enter_context(tc.tile_pool(name="ppool", bufs=1, space="PSUM"))
    spool = ctx.enter_context(tc.tile_pool(name="spool", bufs=1))
    pools = (rawp, maskp, xpool, wpool, fpool, ppool, spool)

    bias_t = spool.tile([P, NB], F32, tag="bias")
    for k in range(NB):
        nc.gpsimd.memset(bias_t[:, k : k + 1], -SQ128 * (k / 15.0))

    for _ in range(repeat):
        _emit_pass(tc, pools, bias_t, stats_d, dstats_d,
                   pred_d, target_d)


def _emit_pass(tc, pools, bias_t, stats_d, dstats_d, pred_d, target_d):
    nc = tc.nc
    add = mybir.AluOpType.add
    mult = mybir.AluOpType.mult
    is_le = mybir.AluOpType.is_le
    DERF = mybir.ActivationFunctionType.Derivative_Erf
    EXP = mybir.ActivationFunctionType.Exp
    rawp, maskp, xpool, wpool, fpool, ppool, spool = pools

    def chan_ap(dram, c):
        return dram[c].rearrange("(q g) -> q g", q=P)

    srcs = [chan_ap(target_d, 0), chan_ap(target_d, 1), chan_ap(target_d, 2),
            chan_ap(pred_d, 0), chan_ap(pred_d, 1), chan_ap(pred_d, 2)]

    raw = []
    for c in range(NCH):
        t = rawp.tile([P, FP], F32, tag=f"raw{c}")
        nc.sync.dma_start(out=t[:], in_=srcs[c])
        raw.append(t)

    # mask offset: off2 = (t0+t1+t2 <= 1.2) ? -4.0 : 0.0
    ts = maskp.tile([P, FP], F32, tag="ts")
    off2 = maskp.tile([P, FP], F32, tag="off2")
    nc.vector.tensor_tensor(out=ts[:], in0=raw[0][:], in1=raw[1][:], op=add)
    nc.vector.tensor_tensor(out=ts[:], in0=ts[:], in1=raw[2][:], op=add)
    nc.vector.tensor_scalar(out=off2[:], in0=ts[:], scalar1=1.2, scalar2=OFF,
                            op0=is_le, op1=mult)

    wo = ppool.tile([P, FP], F32, tag="wo")  # discard activation output
    stats_t = spool.tile([P, NCH * NB], F32)
    dstats_t = spool.tile([P, NCH * L_CHAIN], F32)

    def emit_xp(c):
        xp = xpool.tile([P, FP], F32, tag="xp")
        nc.vector.tensor_tensor(out=xp[:], in0=raw[c][:], in1=off2[:], op=add)
        return xp

    xps = [emit_xp(0)]

    for c in range(NCH):
        xp = xps[c]
        w = wpool.tile([P, FP], F32, tag="w")
        nc.scalar.activation(out=w[:], in_=xp[:], func=EXP,
                             bias=0.0, scale=W_SCALE)
        f_prev = fpool.tile([P, FP], F32, tag="f")
        nc.scalar.activation(out=f_prev[:], in_=xp[:], func=DERF,
                             bias=bias_t[:, 0:1], scale=SQ128,
                             accum_out=stats_t[:, c * NB : c * NB + 1])
        for k in range(1, L_CHAIN + 1):
            f = fpool.tile([P, FP], F32, tag="f")
            col = c * L_CHAIN + (k - 1)
            nc.vector.scalar_tensor_tensor(
                out=f[:], in0=f_prev[:], scalar=BETA[k], in1=w[:],
                op0=mult, op1=mult,
                accum_out=dstats_t[:, col : col + 1])
            f_prev = f
            if k == XP_POS and c + 1 < NCH:
                xps.append(emit_xp(c + 1))
        for k in range(L_CHAIN + 1, NB):
            nc.scalar.activation(out=wo[:], in_=xp[:], func=DERF,
                                 bias=bias_t[:, k : k + 1], scale=SQ128,
                                 accum_out=stats_t[:, c * NB + k : c * NB + k + 1])

    nc.sync.dma_start(out=stats_d[:], in_=stats_t[:])
    nc.sync.dma_start(out=dstats_d[:], in_=dstats_t[:])


def build_nc(repeat=1):
    nc = bacc.Bacc(
        "TRN2", target_bir_lowering=False, debug=False, num_devices=N_CORES
    )
    pred = nc.dram_tensor("pred", [C, HW], F32, kind="ExternalInput").ap()
    target = nc.dram_tensor("target", [C, HW], F32, kind="ExternalInput").ap()
    stats = nc.dram_tensor("stats", [P, NCH * NB], F32, kind="ExternalOutput").ap()
    dstats = nc.dram_tensor("dstats", [P, NCH * L_CHAIN], F32,
                            kind="ExternalOutput").ap()
    with tile.TileContext(nc) as tc:
        with ExitStack() as ctx:
            _kernel_body(ctx, tc, stats, dstats, pred, target, repeat=repeat)
    nc.compile()
    return nc


_NC_CACHE = {}


def _get_nc():
    if "nc" not in _NC_CACHE:
        _NC_CACHE["nc"] = build_nc()
    return _NC_CACHE["nc"]


def stats_to_hists(stats, dstats):
    """per-core [128,96]+[128,42] partials -> hist [2, C, NB] f64 (true exp sums)."""
    s = stats.astype(np.float64).sum(axis=0).reshape(NCH, NB)
    d = dstats.astype(np.float64).sum(axis=0).reshape(NCH, L_CHAIN)
    s[:, 1 : L_CHAIN + 1] = d
    s /= GAUSS_CONST
    hist = np.empty((2, C, NB), np.float64)
    hist[1] = s[0:3]   # target channels 0,1,2
    hist[0] = s[3:6]   # pred channels 0,1,2
    return hist


def finish_on_host(stats_list):
    """list of (stats, dstats) per core -> scalar f32 loss."""
    diffs = []
    for stats, dstats in stats_list:
        hist = stats_to_hists(stats, dstats)
        hist_n = hist / (hist.sum(axis=-1, keepdims=True) + 1e-7)
        diffs.append(np.abs(hist_n[0] - hist_n[1]))
    return np.array(np.mean(np.stack(diffs)), dtype=np.float32)


def run(pred, target, **spmd_kwargs):
    nc = _get_nc()
    pred = np.ascontiguousarray(np.asarray(pred, dtype=np.float32))
    target = np.ascontiguousarray(np.asarray(target, dtype=np.float32))
    assert pred.shape == (N_CORES, C, H, W), pred.shape
    in_maps = [
        {
            "pred": pred[b].reshape(C, HW),
            "target": target[b].reshape(C, HW),
        }
        for b in range(N_CORES)
    ]
    res = run_bass_kernel_spmd(nc, in_maps, core_ids=list(range(N_CORES)), **spmd_kwargs)
    loss = finish_on_host(
        [(res.results[b]["stats"], res.results[b]["dstats"]) for b in range(N_CORES)]
    )
    return loss, res


def kernel(pred, target):
    loss, _ = run(pred, target)
    return loss
